# revision 8
# baseline (speedup 1.0000x reference)
"""Distance-aware comb-pilot interpolator for Trainium2 (8 NeuronCores).

Math: out[b, i, c] = (w_l[i] * H[b, j0(i), c] + w_r[i] * H[b, j1(i), c]) / w[i]
with pilots on the comb loc[k] = 8k (k = 0..511), Nfft = 4096.
For i = 8k + r (k < 511): j0 = k, j1 = k + 1 and the normalized weights
depend only on r:  alpha[r] = w_l/w, gamma[r] = w_r/w, so
  out[:, k, r, :] = alpha[r] * H[:, k, :] + gamma[r] * H[:, k+1, :].
For the last 8 subcarriers the reference extrapolates a virtual pilot
hN = (15/8)H[511] - (7/8)H[510] at subcarrier 4095; folding it in gives
per-(r,c) coefficients on H[510] and H[511] directly ("last-16" columns).

All coefficients depend only on decay = softplus(decay_param), are O(8)
host work, and ship as one tiny [128, 64] constant tile.

Kernel is HBM-store-bound: 2.1 MB in / 16.8 MB out per core; the store
stream drains at ~420 GB/s (~40 us).  Measured structure (fast exec
~72 us total): DVE is the saturated spine (48 scalar_tensor_tensor ops,
~42 us busy, window ~12.5-53 us); ACT produces the scaled copies it
consumes (~27 us, finished by 36 us); stores drain the backlog until
~64 us; the runtime NEFF wrapper adds a fixed ~8.5 us tail (it zeroes
all 253 semaphores one-by-one, split across engines — not controllable
from the kernel).  Design choices that got here:
(a) the first store chunk (pilots 0..128) is gated only on a 258-column
    head slice of tile 0 plus ct, loaded first on the Sync HWDGE ring;
(b) a dependency-free dummy ACTIVATE pulls the ~1.3 us activation-table
    load to ~6.6 us (otherwise the scheduler hoists the first tmp op's
    h0a wait above it, delaying ACT's start by ~5 us);
(c) the r <-> 8-r symmetry of the weights (w(r) = w(8-r), so
    gamma[r] = alpha[8-r]) lets FIVE tmp families tmp[j] = gamma[j]*H
    (j = 0..4) serve all eight r values:
      out[k, j]   = alpha[j]*H[k]   + tmp[j][k+1]
      out[k, 8-j] = alpha[j]*H[k+1] + tmp[j][k]
    cutting ACT work by 3/8 vs one tmp per r;
(d) GpSimd does only the SWDGE bulk loads and the 16 last-subcarrier
    columns: its tensor_tensor ops measure ~2-6 us each (0.42 impl
    efficiency + broadcast operands), so routing any (tile, r) slice
    there makes it the critical path (measured, rejected);
(e) bf16 cannot help: the output must be stored as f32 and only plain
    TENSOR_TENSOR has a 2x DVE mode — scalar_tensor_tensor and
    tensor_scalar run 1x regardless of dtype.
"""

import sys

import numpy as np

for _p in ("/opt/trn_rl_repo", "/root/.axon_site/_ro/trn_rl_repo"):
    if _p not in sys.path:
        sys.path.append(_p)

import concourse.bass as bass
import concourse.tile as tile
from concourse import bacc, mybir
from concourse.bass_utils import run_bass_kernel_spmd

N_CORES = 8
B, NP, NFFT, SPACING = 4096, 512, 4096, 8
B_LOC = B // N_CORES  # batch rows per core
NSEG = NP - 1  # regular 8-wide segments (k = 0..510)
P = 128  # SBUF partitions
N_BT = B_LOC // P  # 128-batch tiles per core
H0A = 258  # columns of the tile-0 head slice (pilots k <= 128, both c)

# Store chunks per tile: (k0, k1) half-open pilot-segment ranges. The final
# chunk of each tile also carries the last-16 output columns. Head chunks are
# small (prime the store stream early), middle chunks big (amortize per-op
# fixed cost), tail chunks small (short post-compute drain).
CHUNKS = {
    0: [(0, 128), (128, NSEG)],
    1: [(0, NSEG)],
    2: [(0, NSEG)],
    3: [(0, 384), (384, NSEG)],
}
# ACT tmp op ranges per tile (decoupled from chunks; full-range where the
# chunk boundary doesn't need it).
ACT_RANGES = {
    0: [(0, 128), (128, NSEG)],
    1: [(0, NSEG)],
    2: [(0, NSEG)],
    3: [(0, NSEG)],
}
# (tile, r) slices computed wholly on GpSimd in u-form. Empirically a dead
# end: Pool has no TensorScalarPtr and its tensor_tensor runs ~4-6 us per
# full-range op (0.42 efficiency + broadcast operands), so any slice routed
# here becomes the critical path. Keep empty; GpSimd does loads + last-16.
PL_SLICES = set()

_PROGRAM = None


def _build_program():
    """One Bass program, identical on all cores (pure data parallel)."""
    nc = bacc.Bacc("TRN2", target_bir_lowering=False, debug=False)
    f32 = mybir.dt.float32
    ls = nc.dram_tensor("ls", [B_LOC, NP * 2], f32, kind="ExternalInput").ap()
    coef = nc.dram_tensor("coef", [P, 64], f32, kind="ExternalInput").ap()
    out = nc.dram_tensor("out", [B_LOC, NFFT * 2], f32, kind="ExternalOutput").ap()

    mult, add = mybir.AluOpType.mult, mybir.AluOpType.add

    with tile.TileContext(nc) as tc:
        with (
            tc.tile_pool(name="cpool", bufs=1) as cpool,
            tc.tile_pool(name="hpool", bufs=4) as hpool,
            tc.tile_pool(name="opool", bufs=3) as opool,
            tc.tile_pool(name="tpool", bufs=10) as tpool,
            tc.tile_pool(name="upool", bufs=2) as upool,
            tc.tile_pool(name="lpool", bufs=2) as lpool,
        ):
            # --- ACT warm-up -----------------------------------------------
            # The activation table load (~1.3 us) is inserted before ACT's
            # first ACTIVATE.  Without this dummy, that is the first tmp op,
            # whose h0a wait the scheduler hoists above the table load —
            # serializing wait + table + op (~5 us late start in traces).
            # A dependency-free dummy ACTIVATE pulls the table load to ~6.6.
            dummy = cpool.tile([P, 1], f32)
            nc.gpsimd.memset(dummy[:], 0.0)
            nc.scalar.mul(dummy[:], dummy[:], 1.0)

            # --- loads -----------------------------------------------------
            # h0a + ct on the Sync HWDGE ring (earliest possible trigger,
            # off the ACT/gpsimd startup paths), h0a first — it gates the
            # first ACT op; the bulk on gpsimd SWDGE so the store ring
            # stays clear.
            h0a = hpool.tile([P, H0A], f32, name="h0a", tag="h0a")
            nc.sync.dma_start(h0a[:], ls[0:P, 0:H0A])
            ct = cpool.tile([P, 64], f32)
            nc.sync.dma_start(ct[:], coef)
            h0b = hpool.tile([P, NP * 2 - 256], f32, name="h0b", tag="h0b")
            nc.gpsimd.dma_start(h0b[:], ls[0:P, 256:])
            hs = [
                (h0a, h0b) if t == 0
                else hpool.tile([P, NP * 2], f32, name="h", tag="h")
                for t in range(N_BT)
            ]
            for t in range(1, N_BT):
                nc.gpsimd.dma_start(hs[t][:], ls[t * P : (t + 1) * P, :])

            def hcols(t, c0, c1):
                """AP over h columns [c0, c1) of tile t (handles split h0)."""
                if t == 0:
                    if c1 <= H0A:
                        return h0a[:, c0:c1]
                    assert c0 >= 256, (c0, c1)
                    return h0b[:, c0 - 256 : c1 - 256]
                return hs[t][:, c0:c1]

            def hseg(t, k0, k1):
                """[P, k1-k0, 2] view of pilots k0..k1 of tile t."""
                return hcols(t, 2 * k0, 2 * k1).rearrange("p (k c) -> p k c", c=2)

            def emit_last16(t, o):
                """Last-16 output columns on GpSimd (3 tiny ops, early)."""
                h510 = hcols(t, 2 * NP - 4, 2 * NP - 2).unsqueeze(1).broadcast_to((P, 8, 2))
                h511 = hcols(t, 2 * NP - 2, 2 * NP).unsqueeze(1).broadcast_to((P, 8, 2))
                a_last = ct[:, 16:32].rearrange("p (r c) -> p r c", c=2)
                c_last = ct[:, 32:48].rearrange("p (r c) -> p r c", c=2)
                tl = lpool.tile([P, 8, 2], f32)
                nc.gpsimd.tensor_mul(tl[:], h510, a_last)
                t2 = lpool.tile([P, 8, 2], f32)
                nc.gpsimd.tensor_mul(t2[:], h511, c_last)
                o_last = o[:, NSEG * 16 : NFFT * 2].rearrange("p (r c) -> p r c", c=2)
                nc.gpsimd.tensor_add(o_last, tl[:], t2[:])

            def ct_bc(col):
                """[P, NSEG, 2] stride-0 broadcast of ct column `col`."""
                return ct[:, col : col + 1].unsqueeze(1).broadcast_to((P, NSEG, 2))

            def emit_pl_slice(t, r, ov):
                """(t, r) in u-form wholly on GpSimd: u = rho*H[k] + H[k+1];
                out = gamma * u.  rho = e^{d(8-2r)} <= 1 for the r >= 4
                slices routed here, so the form is well-conditioned."""
                m = upool.tile([P, NSEG, 2], f32, name="um", tag="um")
                nc.gpsimd.tensor_mul(m[:], hseg(t, 0, NSEG), ct_bc(48 + r))
                u = upool.tile([P, NSEG, 2], f32, name="u", tag="u")
                nc.gpsimd.tensor_add(u[:], m[:], hseg(t, 1, NSEG + 1))
                nc.gpsimd.tensor_mul(ov[:, 0:NSEG, r, :], u[:], ct_bc(8 + r))

            # --- main pipeline --------------------------------------------
            os_ = []
            for t in range(N_BT):
                o = opool.tile([P, NFFT * 2], f32)
                os_.append(o)
            ovs = [
                os_[t][:].rearrange("p (k r c) -> p k r c", r=SPACING, c=2)
                for t in range(N_BT)
            ]

            for t in range(N_BT):
                o, ov = os_[t], ovs[t]
                act_rs = [r for r in range(SPACING) if (t, r) not in PL_SLICES]
                pl_rs = [r for r in range(SPACING) if (t, r) in PL_SLICES]

                # GpSimd work for this tile, emitted before the ACT/DVE sweep
                # so it's never the chunk-store gate.
                emit_last16(t, o)
                for r in pl_rs:
                    emit_pl_slice(t, r, ov)

                # ACT tmp spine, pair-folded: w is symmetric in r <-> 8-r, so
                # gamma[r] = alpha[8-r] and one family tmp[j] = gamma[j]*H
                # (j = 0..4) serves BOTH r = j and r = 8-j:
                #   out[k, j]   = alpha[j]*H[k]   + tmp[j][k+1]
                #   out[k, 8-j] = alpha[j]*H[k+1] + tmp[j][k]
                # Five ACT copies per range instead of eight. tmp[j] spans
                # pilots (m0, m1+1) to cover both the k and k+1 uses.
                tmps = {}
                for m0, m1 in ACT_RANGES[t]:
                    for j in range(5):
                        tmp = tpool.tile([P, NP, 2], f32, name="tmp", tag="tmp")
                        nc.scalar.mul(
                            tmp[:, 0 : m1 + 1 - m0, :],
                            hseg(t, m0, m1 + 1),
                            ct[:, 8 + j : 9 + j],
                        )
                        tmps[(m0, j)] = tmp

                # DVE combine + chunk stores
                for k0, k1 in CHUNKS[t]:
                    last = k1 == NSEG
                    for r in act_rs:
                        j = r if r <= 4 else 8 - r
                        m0, m1 = next(
                            m for m in ACT_RANGES[t] if m[0] <= k0 and k1 <= m[1]
                        )
                        tv = tmps[(m0, j)]
                        if r <= 4:
                            in0 = hseg(t, k0, k1)
                            in1 = tv[:, k0 + 1 - m0 : k1 + 1 - m0, :]
                        else:
                            in0 = hseg(t, k0 + 1, k1 + 1)
                            in1 = tv[:, k0 - m0 : k1 - m0, :]
                        nc.vector.scalar_tensor_tensor(
                            ov[:, k0:k1, r, :],
                            in0,
                            ct[:, j : j + 1],
                            in1,
                            mult,
                            add,
                        )
                    lo = k0 * 16
                    hi = NFFT * 2 if last else k1 * 16
                    nc.sync.dma_start(out[t * P : (t + 1) * P, lo:hi], o[:, lo:hi])
    nc.compile()
    return nc


def _coef_tile(decay_param: np.ndarray) -> np.ndarray:
    """[128, 64] f32: cols 0:8 alpha[r], 8:16 gamma[r], 16:32 last-chunk
    coeff on H[510] (r,c-flattened), 32:48 last-chunk coeff on H[511],
    48:56 rho[r] = alpha[r]/gamma[r]."""
    x = np.float32(np.asarray(decay_param).reshape(-1)[0])
    d = np.logaddexp(np.float32(0.0), x, dtype=np.float32)  # softplus
    r = np.arange(SPACING, dtype=np.float32)
    eps = np.float32(1e-12)
    # regular segments: x1 - x0 = 8
    wl = np.exp(-d * r, dtype=np.float32)
    wr = np.exp(-d * (np.float32(SPACING) - r), dtype=np.float32)
    w = wl + wr + eps
    alpha, gamma = wl / w, wr / w
    # last chunk: i = 4088 + r, x0 = 4088, x1 = 4095 (gap of 7);
    # y1 = hN = (15/8) H[511] - (7/8) H[510]
    wl2 = np.exp(-d * r, dtype=np.float32)
    wr2 = np.exp(-d * (np.float32(7.0) - r), dtype=np.float32)
    w2 = wl2 + wr2 + eps
    c511 = (wl2 + np.float32(1.875) * wr2) / w2
    c510 = -np.float32(0.875) * wr2 / w2
    # rho = alpha/gamma = exp(d*(8-2r)); used only for the r >= 4 slices
    # computed in u-form, where rho <= 1, but keep the guard for tiny gamma.
    rho = np.clip(alpha / np.maximum(gamma, np.float32(1e-30)), 0, 3.0e38).astype(
        np.float32
    )
    row = np.concatenate(
        [alpha, gamma, np.repeat(c510, 2), np.repeat(c511, 2),
         rho, np.zeros(8, np.float32)]
    ).astype(np.float32)
    return np.broadcast_to(row, (P, 64)).copy()


def kernel(LS_ri, pilot_pos=None, decay_param=None, Nfft=None, **_unused):
    global _PROGRAM
    LS_ri = np.ascontiguousarray(np.asarray(LS_ri, dtype=np.float32))
    coef = _coef_tile(decay_param)

    if _PROGRAM is None:
        _PROGRAM = _build_program()
    nc = _PROGRAM

    in_maps = []
    for c in range(N_CORES):
        shard = LS_ri[c * B_LOC : (c + 1) * B_LOC].reshape(B_LOC, NP * 2)
        in_maps.append({"ls": shard, "coef": coef})

    res = run_bass_kernel_spmd(nc, in_maps, list(range(N_CORES))).results
    out = np.concatenate(
        [res[c]["out"].reshape(B_LOC, NFFT, 2) for c in range(N_CORES)], axis=0
    )
    return out


# revision 11
# speedup vs baseline: 1.0001x; 1.0001x over previous
"""Distance-aware comb-pilot interpolator for Trainium2 (8 NeuronCores).

Math: out[b, i, c] = (w_l[i] * H[b, j0(i), c] + w_r[i] * H[b, j1(i), c]) / w[i]
with pilots on the comb loc[k] = 8k (k = 0..511), Nfft = 4096.
For i = 8k + r (k < 511): j0 = k, j1 = k + 1 and the normalized weights
depend only on r:  alpha[r] = w_l/w, gamma[r] = w_r/w, so
  out[:, k, r, :] = alpha[r] * H[:, k, :] + gamma[r] * H[:, k+1, :].
For the last 8 subcarriers the reference extrapolates a virtual pilot
hN = (15/8)H[511] - (7/8)H[510] at subcarrier 4095; folding it in gives
per-(r,c) coefficients on H[510] and H[511] directly ("last-16" columns).

All coefficients depend only on decay = softplus(decay_param), are O(8)
host work, and ship as one tiny [128, 64] constant tile.

Kernel is HBM-store-bound: 2.1 MB in / 16.8 MB out per core; the store
stream drains at ~420 GB/s (~40 us).  Measured structure (fast exec
~72 us total): DVE is the saturated spine (48 scalar_tensor_tensor ops,
~42 us busy, window ~12.5-53 us); ACT produces the scaled copies it
consumes (~27 us, finished by 36 us); stores drain the backlog until
~64 us; the runtime NEFF wrapper adds a fixed ~8.5 us tail (it zeroes
all 253 semaphores one-by-one, split across engines — not controllable
from the kernel).  Design choices that got here:
(a) the first store chunk (pilots 0..128) is gated only on a 258-column
    head slice of tile 0 plus ct, loaded first on the Sync HWDGE ring;
(b) a dependency-free dummy ACTIVATE pulls the ~1.3 us activation-table
    load to ~6.6 us (otherwise the scheduler hoists the first tmp op's
    h0a wait above it, delaying ACT's start by ~5 us);
(c) the r <-> 8-r symmetry of the weights (w(r) = w(8-r), so
    gamma[r] = alpha[8-r]) lets FIVE tmp families tmp[j] = gamma[j]*H
    (j = 0..4) serve all eight r values:
      out[k, j]   = alpha[j]*H[k]   + tmp[j][k+1]
      out[k, 8-j] = alpha[j]*H[k+1] + tmp[j][k]
    cutting ACT work by 3/8 vs one tmp per r;
(d) GpSimd does only the SWDGE bulk loads and the 16 last-subcarrier
    columns: its tensor_tensor ops measure ~2-6 us each (0.42 impl
    efficiency + broadcast operands), so routing any (tile, r) slice
    there makes it the critical path (measured, rejected);
(e) bf16 cannot help: the output must be stored as f32 and only plain
    TENSOR_TENSOR has a 2x DVE mode — scalar_tensor_tensor and
    tensor_scalar run 1x regardless of dtype.
"""

import sys

import numpy as np

for _p in ("/opt/trn_rl_repo", "/root/.axon_site/_ro/trn_rl_repo"):
    if _p not in sys.path:
        sys.path.append(_p)

import concourse.bass as bass
import concourse.tile as tile
from concourse import bacc, mybir
from concourse.bass_utils import run_bass_kernel_spmd

N_CORES = 8
B, NP, NFFT, SPACING = 4096, 512, 4096, 8
B_LOC = B // N_CORES  # batch rows per core
NSEG = NP - 1  # regular 8-wide segments (k = 0..510)
P = 128  # SBUF partitions
N_BT = B_LOC // P  # 128-batch tiles per core
H0A = 258  # columns of the tile-0 head slice (pilots k <= 128, both c)

# Store chunks per tile: (k0, k1) half-open pilot-segment ranges. The final
# chunk of each tile also carries the last-16 output columns. Head chunks are
# small (prime the store stream early), middle chunks big (amortize per-op
# fixed cost), tail chunks small (short post-compute drain).
CHUNKS = {
    0: [(0, 128), (128, NSEG)],
    1: [(0, NSEG)],
    2: [(0, NSEG)],
    3: [(0, 384), (384, 448), (448, NSEG)],
}
# ACT tmp op ranges per tile (decoupled from chunks; full-range where the
# chunk boundary doesn't need it).
ACT_RANGES = {
    0: [(0, 128), (128, NSEG)],
    1: [(0, NSEG)],
    2: [(0, NSEG)],
    3: [(0, NSEG)],
}
# (tile, r) slices computed wholly on GpSimd in u-form. Empirically a dead
# end: Pool has no TensorScalarPtr and its tensor_tensor runs ~4-6 us per
# full-range op (0.42 efficiency + broadcast operands), so any slice routed
# here becomes the critical path. Keep empty; GpSimd does loads + last-16.
PL_SLICES = set()

_PROGRAM = None


def _build_program():
    """One Bass program, identical on all cores (pure data parallel)."""
    nc = bacc.Bacc("TRN2", target_bir_lowering=False, debug=False)
    f32 = mybir.dt.float32
    ls = nc.dram_tensor("ls", [B_LOC, NP * 2], f32, kind="ExternalInput").ap()
    coef = nc.dram_tensor("coef", [P, 64], f32, kind="ExternalInput").ap()
    out = nc.dram_tensor("out", [B_LOC, NFFT * 2], f32, kind="ExternalOutput").ap()

    mult, add = mybir.AluOpType.mult, mybir.AluOpType.add

    with tile.TileContext(nc) as tc:
        with (
            tc.tile_pool(name="cpool", bufs=1) as cpool,
            tc.tile_pool(name="hpool", bufs=4) as hpool,
            tc.tile_pool(name="opool", bufs=3) as opool,
            tc.tile_pool(name="tpool", bufs=10) as tpool,
            tc.tile_pool(name="upool", bufs=2) as upool,
            tc.tile_pool(name="lpool", bufs=2) as lpool,
        ):
            # --- loads -----------------------------------------------------
            # ct + h0a gate the first ACT op, and every ns here moves the
            # total 1:1 (the DVE chain + store drain hang off h0a-ready,
            # measured ~11.9 us = trigger 7.2 + 1 us transfer at 132 GB/s
            # on 1 KB descriptors + ~2.2 us completion receipt).  ct is
            # tiny — load it first; h0a is split across BOTH HWDGE rings
            # (sync + scalar) to halve its serial transfer.  Bulk loads on
            # gpsimd SWDGE keep the store ring clear.
            dummy = cpool.tile([P, 1], f32)
            nc.gpsimd.memset(dummy[:], 0.0)
            ct = cpool.tile([P, 64], f32)
            nc.sync.dma_start(ct[:], coef)
            h0a = hpool.tile([P, H0A], f32, name="h0a", tag="h0a")
            nc.sync.dma_start(h0a[:, 0:130], ls[0:P, 0:130])
            nc.scalar.dma_start(h0a[:, 130:H0A], ls[0:P, 130:H0A])

            # --- ACT warm-up -----------------------------------------------
            # The activation table load (~1.3 us) is inserted before ACT's
            # first ACTIVATE.  Without this dummy, that is the first tmp op,
            # whose h0a wait the scheduler hoists above the table load —
            # serializing wait + table + op (~5 us late start in traces).
            # A dependency-free dummy ACTIVATE pulls the table load into the
            # preamble shadow; it sits AFTER the scalar-ring h0a trigger so
            # that DMA fires before the table load occupies the engine.
            nc.scalar.mul(dummy[:], dummy[:], 1.0)
            h0b = hpool.tile([P, NP * 2 - 256], f32, name="h0b", tag="h0b")
            nc.gpsimd.dma_start(h0b[:], ls[0:P, 256:])
            hs = [
                (h0a, h0b) if t == 0
                else hpool.tile([P, NP * 2], f32, name="h", tag="h")
                for t in range(N_BT)
            ]
            for t in range(1, N_BT):
                nc.gpsimd.dma_start(hs[t][:], ls[t * P : (t + 1) * P, :])

            def hcols(t, c0, c1):
                """AP over h columns [c0, c1) of tile t (handles split h0)."""
                if t == 0:
                    if c1 <= H0A:
                        return h0a[:, c0:c1]
                    assert c0 >= 256, (c0, c1)
                    return h0b[:, c0 - 256 : c1 - 256]
                return hs[t][:, c0:c1]

            def hseg(t, k0, k1):
                """[P, k1-k0, 2] view of pilots k0..k1 of tile t."""
                return hcols(t, 2 * k0, 2 * k1).rearrange("p (k c) -> p k c", c=2)

            def emit_last16(t, o):
                """Last-16 output columns on GpSimd (3 tiny ops, early)."""
                h510 = hcols(t, 2 * NP - 4, 2 * NP - 2).unsqueeze(1).broadcast_to((P, 8, 2))
                h511 = hcols(t, 2 * NP - 2, 2 * NP).unsqueeze(1).broadcast_to((P, 8, 2))
                a_last = ct[:, 16:32].rearrange("p (r c) -> p r c", c=2)
                c_last = ct[:, 32:48].rearrange("p (r c) -> p r c", c=2)
                tl = lpool.tile([P, 8, 2], f32)
                nc.gpsimd.tensor_mul(tl[:], h510, a_last)
                t2 = lpool.tile([P, 8, 2], f32)
                nc.gpsimd.tensor_mul(t2[:], h511, c_last)
                o_last = o[:, NSEG * 16 : NFFT * 2].rearrange("p (r c) -> p r c", c=2)
                nc.gpsimd.tensor_add(o_last, tl[:], t2[:])

            def ct_bc(col):
                """[P, NSEG, 2] stride-0 broadcast of ct column `col`."""
                return ct[:, col : col + 1].unsqueeze(1).broadcast_to((P, NSEG, 2))

            def emit_pl_slice(t, r, ov):
                """(t, r) in u-form wholly on GpSimd: u = rho*H[k] + H[k+1];
                out = gamma * u.  rho = e^{d(8-2r)} <= 1 for the r >= 4
                slices routed here, so the form is well-conditioned."""
                m = upool.tile([P, NSEG, 2], f32, name="um", tag="um")
                nc.gpsimd.tensor_mul(m[:], hseg(t, 0, NSEG), ct_bc(48 + r))
                u = upool.tile([P, NSEG, 2], f32, name="u", tag="u")
                nc.gpsimd.tensor_add(u[:], m[:], hseg(t, 1, NSEG + 1))
                nc.gpsimd.tensor_mul(ov[:, 0:NSEG, r, :], u[:], ct_bc(8 + r))

            # --- main pipeline --------------------------------------------
            os_ = []
            for t in range(N_BT):
                o = opool.tile([P, NFFT * 2], f32)
                os_.append(o)
            ovs = [
                os_[t][:].rearrange("p (k r c) -> p k r c", r=SPACING, c=2)
                for t in range(N_BT)
            ]

            for t in range(N_BT):
                o, ov = os_[t], ovs[t]
                act_rs = [r for r in range(SPACING) if (t, r) not in PL_SLICES]
                pl_rs = [r for r in range(SPACING) if (t, r) in PL_SLICES]

                # GpSimd work for this tile, emitted before the ACT/DVE sweep
                # so it's never the chunk-store gate.
                emit_last16(t, o)
                for r in pl_rs:
                    emit_pl_slice(t, r, ov)

                # ACT tmp spine, pair-folded: w is symmetric in r <-> 8-r, so
                # gamma[r] = alpha[8-r] and one family tmp[j] = gamma[j]*H
                # (j = 0..4) serves BOTH r = j and r = 8-j:
                #   out[k, j]   = alpha[j]*H[k]   + tmp[j][k+1]
                #   out[k, 8-j] = alpha[j]*H[k+1] + tmp[j][k]
                # Five ACT copies per range instead of eight. tmp[j] spans
                # pilots (m0, m1+1) to cover both the k and k+1 uses.
                tmps = {}
                for m0, m1 in ACT_RANGES[t]:
                    for j in range(5):
                        tmp = tpool.tile([P, NP, 2], f32, name="tmp", tag="tmp")
                        nc.scalar.mul(
                            tmp[:, 0 : m1 + 1 - m0, :],
                            hseg(t, m0, m1 + 1),
                            ct[:, 8 + j : 9 + j],
                        )
                        tmps[(m0, j)] = tmp

                # DVE combine + chunk stores
                for k0, k1 in CHUNKS[t]:
                    last = k1 == NSEG
                    for r in act_rs:
                        j = r if r <= 4 else 8 - r
                        m0, m1 = next(
                            m for m in ACT_RANGES[t] if m[0] <= k0 and k1 <= m[1]
                        )
                        tv = tmps[(m0, j)]
                        if r <= 4:
                            in0 = hseg(t, k0, k1)
                            in1 = tv[:, k0 + 1 - m0 : k1 + 1 - m0, :]
                        else:
                            in0 = hseg(t, k0 + 1, k1 + 1)
                            in1 = tv[:, k0 - m0 : k1 - m0, :]
                        nc.vector.scalar_tensor_tensor(
                            ov[:, k0:k1, r, :],
                            in0,
                            ct[:, j : j + 1],
                            in1,
                            mult,
                            add,
                        )
                    lo = k0 * 16
                    hi = NFFT * 2 if last else k1 * 16
                    nc.sync.dma_start(out[t * P : (t + 1) * P, lo:hi], o[:, lo:hi])
    nc.compile()
    return nc


def _coef_tile(decay_param: np.ndarray) -> np.ndarray:
    """[128, 64] f32: cols 0:8 alpha[r], 8:16 gamma[r], 16:32 last-chunk
    coeff on H[510] (r,c-flattened), 32:48 last-chunk coeff on H[511],
    48:56 rho[r] = alpha[r]/gamma[r]."""
    x = np.float32(np.asarray(decay_param).reshape(-1)[0])
    d = np.logaddexp(np.float32(0.0), x, dtype=np.float32)  # softplus
    r = np.arange(SPACING, dtype=np.float32)
    eps = np.float32(1e-12)
    # regular segments: x1 - x0 = 8
    wl = np.exp(-d * r, dtype=np.float32)
    wr = np.exp(-d * (np.float32(SPACING) - r), dtype=np.float32)
    w = wl + wr + eps
    alpha, gamma = wl / w, wr / w
    # last chunk: i = 4088 + r, x0 = 4088, x1 = 4095 (gap of 7);
    # y1 = hN = (15/8) H[511] - (7/8) H[510]
    wl2 = np.exp(-d * r, dtype=np.float32)
    wr2 = np.exp(-d * (np.float32(7.0) - r), dtype=np.float32)
    w2 = wl2 + wr2 + eps
    c511 = (wl2 + np.float32(1.875) * wr2) / w2
    c510 = -np.float32(0.875) * wr2 / w2
    # rho = alpha/gamma = exp(d*(8-2r)); used only for the r >= 4 slices
    # computed in u-form, where rho <= 1, but keep the guard for tiny gamma.
    rho = np.clip(alpha / np.maximum(gamma, np.float32(1e-30)), 0, 3.0e38).astype(
        np.float32
    )
    row = np.concatenate(
        [alpha, gamma, np.repeat(c510, 2), np.repeat(c511, 2),
         rho, np.zeros(8, np.float32)]
    ).astype(np.float32)
    return np.broadcast_to(row, (P, 64)).copy()


def kernel(LS_ri, pilot_pos=None, decay_param=None, Nfft=None, **_unused):
    global _PROGRAM
    LS_ri = np.ascontiguousarray(np.asarray(LS_ri, dtype=np.float32))
    coef = _coef_tile(decay_param)

    if _PROGRAM is None:
        _PROGRAM = _build_program()
    nc = _PROGRAM

    in_maps = []
    for c in range(N_CORES):
        shard = LS_ri[c * B_LOC : (c + 1) * B_LOC].reshape(B_LOC, NP * 2)
        in_maps.append({"ls": shard, "coef": coef})

    res = run_bass_kernel_spmd(nc, in_maps, list(range(N_CORES))).results
    out = np.concatenate(
        [res[c]["out"].reshape(B_LOC, NFFT, 2) for c in range(N_CORES)], axis=0
    )
    return out
